# revision 23
# baseline (speedup 1.0000x reference)
"""Trainium2 Bass kernel for nn_CTR_27754078666791 (batched Sinkhorn OT loss).

Reference semantics: 200-iteration Sinkhorn with a convergence check at
t = 0, 50, 100, 150 that freezes the iterates once
    max_b |sum_k u_new*Kv - sum_k a| <= 5e-3.
Because u_new = a/(Kv+eps), the checked quantity is a/(Kv+eps)*Kv ~ a up to
f32 rounding (~1e-4), so the check passes at t=0 for any inputs: the loop
always freezes after ONE Sinkhorn iteration from the uniform init
u0 = 1/K, v0 = 1/V.  The computation therefore reduces to:

    E[v,k]  = exp(-alpha*M[v,k])                  (K_mat transposed)
    s[v]    = sum_k E[v,k] / K                     (= K^T u0, batch-indep)
    v1[b,v] = b[b,v] / (s[v] + eps)
    Kv1     = v1 @ E          [B,K]
    G       = v1 @ (E*M)      [B,K]
    u1      = a / (Kv1 + eps)
    loss    = mean_b sum_k u1[b,k] * G[b,k]

Distribution: shard V=5000 across 8 cores (625 rows of M / cols of b each).
Each core reads only its M/b shard, produces partial Kv1_c / G_c [B,K]
sums; the tiny [64,256] partials are summed on host (the final mean
all-reduce), where u1 and the loss are formed.

Device kernel per core (Tile), final:
  - Host pre-arranges shards into the exact SBUF layout (625 v-rows folded
    into 5 groups of 125 partitions side by side in the free dim) and
    converts to bf16: M [125, 5*256], bT [125, 5*64] (b pre-transposed, so
    no on-device transposes).  bf16 M shifts the loss by ~2e-4 relative
    (verified) -- far inside the tolerance -- and halves the dominant DMA.
    DRAM->SBUF traffic is pinned to 5 SDMA engines (~120 GB/s), so input
    bytes are the scarce resource.
  - Input DMAs ride the SP HWDGE ring in pipeline order (g0, g1, bT,
    g2, g3, g4): a small first chunk starts the ACT chain early, bT rides
    third so v1T never gates the first matmul, and per-group chunks keep
    completion semaphores flowing (per-chunk completion lags data by
    1-2 us under load).
  - Dummy matmuls on a zeroed scratch tile run during the DMA wait (plus
    two fillers mid-stream) to lift the PE HAM clock gate (cold PE runs
    at 1.2 GHz; warm at 2.4 GHz) before the real matmuls.
  - Per group g: ACT does E_g=exp(-20*M_g) with the row-sum fused into the
    activation accumulator (s_g); the E*M product C_g runs on GpSimd for
    early groups and DVE for the last two (critical path); DVE does
    r_g=1/s_g and v1T_g=(bT_g*K)*r_g; PE runs one bf16 matmul
    [Kv1_g|G_g] += v1T_g.T @ [E_g|C_g] into a single [64,512] PSUM bank.
  - One DVE cast PSUM->SBUF (bf16); one DMA back to DRAM; the host sums
    the 8 partials and finishes u1 and the mean loss.
"""

import numpy as np

# Problem constants (hardcoded per harness contract).
B = 64
K = 256
V = 5000
NCORES = 8
VC = V // NCORES  # 625 rows of M per core
P = 125           # partition rows per group
NG = VC // P      # 5 groups per core
GSPLIT = 2        # m DMA chunk A covers groups [0, GSPLIT)
N_WARM = 9         # dummy matmuls to lift the PE HAM clock gate during DMA
ALPHA = 20.0
EPS = 1e-16

_CACHE = {}


def _build_nc():
    from concourse import bacc, mybir, tile

    f32 = mybir.dt.float32
    bf16 = mybir.dt.bfloat16
    Act = mybir.ActivationFunctionType
    Alu = mybir.AluOpType

    nc = bacc.Bacc(
        "TRN2",
        debug=False,
        enable_asserts=False,
        num_devices=NCORES,
    )
    m_d = nc.dram_tensor("m_sh", [P, NG * K], bf16, kind="ExternalInput").ap()
    bt_d = nc.dram_tensor("bt_sh", [P, NG * B], bf16, kind="ExternalInput").ap()
    o_d = nc.dram_tensor("out", [B, 2 * K], bf16, kind="ExternalOutput").ap()

    with tile.TileContext(nc) as tc:
        with (
            tc.tile_pool(name="mt", bufs=1) as mpool,
            tc.tile_pool(name="bt", bufs=1) as btpool,
            tc.tile_pool(name="ec", bufs=1) as ecpool,
            tc.tile_pool(name="v1", bufs=1) as vpool,
            tc.tile_pool(name="sc", bufs=NG) as spool,
            tc.tile_pool(name="osb", bufs=1) as opool,
            tc.tile_pool(name="pacc", bufs=1, space="PSUM") as paccp,
        ):
            m_sb = mpool.tile([P, NG * K], bf16, tag="m")
            bt_sb = btpool.tile([P, NG * B], bf16, tag="bt")
            ec = ecpool.tile([P, NG * 2 * K], bf16, tag="ec")
            v1t = vpool.tile([P, NG * B], bf16, tag="v1t")
            psum = paccp.tile([B, 2 * K], f32, tag="acc")

            scratch = vpool.tile([P, 2 * K], bf16, tag="warm")
            nc.gpsimd.memset(scratch[:], 0.0)
            wpsum = paccp.tile([B, 2 * K], f32, tag="warmps")
            for _ in range(N_WARM):
                nc.tensor.matmul(
                    wpsum[:], scratch[:, 0:B], scratch[:], start=True, stop=True
                )

            m3 = m_sb[:].rearrange("p (g k) -> p g k", g=NG)
            md3 = m_d.rearrange("p (g k) -> p g k", g=NG)
            bt3 = bt_sb[:].rearrange("p (g b) -> p g b", g=NG)
            ec3 = ec[:].rearrange("p (g k) -> p g k", g=NG)
            v3 = v1t[:].rearrange("p (g b) -> p g b", g=NG)

            # Input DMAs in pipeline order on the SP HWDGE ring (input
            # traffic shares 5 SDMA engines; strict FIFO per ring).  Small
            # first chunk starts the ACT chain early; bT rides second so
            # v1T never gates; per-group chunks keep completions flowing.
            nc.sync.dma_start(out=m3[:, 0:1, :], in_=md3[:, 0:1, :])
            nc.sync.dma_start(out=m3[:, 1:2, :], in_=md3[:, 1:2, :])
            nc.sync.dma_start(out=bt_sb[:], in_=bt_d)
            for g in range(2, NG):
                nc.sync.dma_start(out=m3[:, g : g + 1, :], in_=md3[:, g : g + 1, :])

            for g in range(NG):
                # E_g (bf16) with fused row-sum accumulator s_g.
                s = spool.tile([P, 1], f32, tag="s")
                nc.scalar.activation(
                    ec3[:, g, 0:K], m3[:, g, :], Act.Exp, scale=-ALPHA,
                    accum_out=s[:],
                )
                # C_g = E_g * M_g.  GpSimd for early groups (parallel with
                # DVE); DVE for the last group, whose C is on the critical
                # path into the final matmul.
                ceng = nc.vector if g >= NG - 2 else nc.gpsimd
                ceng.tensor_tensor(
                    ec3[:, g, K : 2 * K], ec3[:, g, 0:K], m3[:, g, :],
                    op=Alu.mult,
                )
                # r_g = 1/s_g; v1T_g = (bT_g * K) * r_g.  (The reference's
                # eps=1e-16 on K^T u0 >= 2e-9 is below f32 resolution of
                # the quotient -- dropped.  The 1/K on s folds into the
                # v1T scale.)
                r = spool.tile([P, 1], f32, tag="r")
                nc.vector.reciprocal(r[:], s[:])
                nc.vector.tensor_scalar(
                    v3[:, g, :], bt3[:, g, :], r[:], float(K),
                    op0=Alu.mult, op1=Alu.mult,
                )
                # [Kv1 | G] += v1T_g.T @ [E_g | C_g]
                nc.tensor.matmul(
                    psum[:], v3[:, g, :], ec3[:, g, :],
                    start=(g == 0), stop=(g == NG - 1),
                )
                if g in (1, 2):
                    # filler matmul: keeps the PE HAM clock gate lifted
                    # through the dependency gaps between real matmuls
                    nc.tensor.matmul(
                        wpsum[:], scratch[:, 0:B], scratch[:],
                        start=True, stop=True,
                    )

            out_sb = opool.tile([B, 2 * K], bf16, tag="osb")
            nc.vector.tensor_copy(out_sb[:], psum[:])
            nc.sync.dma_start(out=o_d, in_=out_sb[:])

    nc.compile()
    return nc


def _get_nc():
    if "nc" not in _CACHE:
        _CACHE["nc"] = _build_nc()
    return _CACHE["nc"]


def _shard_host(b, M):
    """Pre-arrange shards into the on-chip layout: group-fold v into
    [125, 5*...] with groups side by side in the free dimension, bf16."""
    import ml_dtypes

    M = np.asarray(M, dtype=np.float32)
    bt = np.asarray(b, dtype=np.float32).T.astype(ml_dtypes.bfloat16)  # [V, B]
    in_maps = []
    for c in range(NCORES):
        lo, hi = c * VC, (c + 1) * VC
        # [VC, K] -> [NG, P, K] -> [P, NG, K] -> [P, NG*K]
        msh = (
            M[lo:hi, :]
            .astype(ml_dtypes.bfloat16)
            .reshape(NG, P, K)
            .transpose(1, 0, 2)
            .reshape(P, NG * K)
        )
        bsh = (
            bt[lo:hi, :].reshape(NG, P, B).transpose(1, 0, 2).reshape(P, NG * B)
        )
        in_maps.append(
            {
                "m_sh": np.ascontiguousarray(msh),
                "bt_sh": np.ascontiguousarray(bsh),
            }
        )
    return in_maps


def run_on_hw(a, b, M, trace=False):
    """Returns (loss, BassKernelResults)."""
    from concourse import bass_utils

    nc = _get_nc()
    res = bass_utils.run_bass_kernel_spmd(
        nc,
        _shard_host(b, M),
        core_ids=list(range(NCORES)),
        trace=trace,
    )
    outs = [res.results[c]["out"] for c in range(NCORES)]
    acc = np.sum(np.stack(outs, axis=0), axis=0)  # [B, 2K]
    kv1 = acc[:, :K]
    g = acc[:, K:]
    u1 = np.asarray(a, dtype=np.float32) / (kv1 + np.float32(EPS))
    loss = np.float32(np.mean(np.sum(u1 * g, axis=1)))
    return np.asarray(loss), res


def kernel(a, b, M):
    loss, _ = run_on_hw(a, b, M, trace=False)
    return loss


# revision 24
# speedup vs baseline: 1.0032x; 1.0032x over previous
"""Trainium2 Bass kernel for nn_CTR_27754078666791 (batched Sinkhorn OT loss).

Reference semantics: 200-iteration Sinkhorn with a convergence check at
t = 0, 50, 100, 150 that freezes the iterates once
    max_b |sum_k u_new*Kv - sum_k a| <= 5e-3.
Because u_new = a/(Kv+eps), the checked quantity is a/(Kv+eps)*Kv ~ a up to
f32 rounding (~1e-4), so the check passes at t=0 for any inputs: the loop
always freezes after ONE Sinkhorn iteration from the uniform init
u0 = 1/K, v0 = 1/V.  The computation therefore reduces to:

    E[v,k]  = exp(-alpha*M[v,k])                  (K_mat transposed)
    s[v]    = sum_k E[v,k] / K                     (= K^T u0, batch-indep)
    v1[b,v] = b[b,v] / (s[v] + eps)
    Kv1     = v1 @ E          [B,K]
    G       = v1 @ (E*M)      [B,K]
    u1      = a / (Kv1 + eps)
    loss    = mean_b sum_k u1[b,k] * G[b,k]

Distribution: shard V=5000 across 8 cores (625 rows of M / cols of b each).
Each core reads only its M/b shard, produces partial Kv1_c / G_c [B,K]
sums; the tiny [64,256] partials are summed on host (the final mean
all-reduce), where u1 and the loss are formed.

Device kernel per core (Tile), final:
  - Host pre-arranges shards into the exact SBUF layout (625 v-rows folded
    into 5 groups of 125 partitions side by side in the free dim) and
    converts to bf16: M [125, 5*256], bT [125, 5*64] (b pre-transposed, so
    no on-device transposes).  bf16 M shifts the loss by ~2e-4 relative
    (verified) -- far inside the tolerance -- and halves the dominant DMA.
    DRAM->SBUF traffic is pinned to 5 SDMA engines (~120 GB/s), so input
    bytes are the scarce resource.
  - Input DMAs ride per-group: SP HWDGE ring carries g0, g1, bT, g3, g4
    in pipeline order while g2 goes alone on the ACT ring (independent
    completion queue, pulls the mid-chain gate ~1us earlier).  Per-chunk
    completion semaphores lag the data by 1-2 us under load, which is why
    chunk order and ring placement matter more than raw bandwidth.
  - Dummy matmuls on a zeroed scratch tile run during the DMA wait (plus
    two fillers mid-stream) to lift the PE HAM clock gate (cold PE runs
    at 1.2 GHz; warm at 2.4 GHz) before the real matmuls.
  - Per group g: ACT does E_g=exp(-20*M_g) with the row-sum fused into the
    activation accumulator (s_g); the E*M product C_g runs on GpSimd
    except group 3 on DVE (critical-path balance); DVE does
    r_g=1/s_g and v1T_g=(bT_g*K)*r_g; PE runs one bf16 matmul
    [Kv1_g|G_g] += v1T_g.T @ [E_g|C_g] into a single [64,512] PSUM bank.
  - One DVE cast PSUM->SBUF (bf16); one DMA back to DRAM; the host sums
    the 8 partials and finishes u1 and the mean loss.
"""

import numpy as np

# Problem constants (hardcoded per harness contract).
B = 64
K = 256
V = 5000
NCORES = 8
VC = V // NCORES  # 625 rows of M per core
P = 125           # partition rows per group
NG = VC // P      # 5 groups per core
GSPLIT = 2        # m DMA chunk A covers groups [0, GSPLIT)
N_WARM = 9         # dummy matmuls to lift the PE HAM clock gate during DMA
ALPHA = 20.0
EPS = 1e-16

_CACHE = {}


def _build_nc():
    from concourse import bacc, mybir, tile

    f32 = mybir.dt.float32
    bf16 = mybir.dt.bfloat16
    Act = mybir.ActivationFunctionType
    Alu = mybir.AluOpType

    nc = bacc.Bacc(
        "TRN2",
        debug=False,
        enable_asserts=False,
        num_devices=NCORES,
    )
    m_d = nc.dram_tensor("m_sh", [P, NG * K], bf16, kind="ExternalInput").ap()
    bt_d = nc.dram_tensor("bt_sh", [P, NG * B], bf16, kind="ExternalInput").ap()
    o_d = nc.dram_tensor("out", [B, 2 * K], bf16, kind="ExternalOutput").ap()

    with tile.TileContext(nc) as tc:
        with (
            tc.tile_pool(name="mt", bufs=1) as mpool,
            tc.tile_pool(name="bt", bufs=1) as btpool,
            tc.tile_pool(name="ec", bufs=1) as ecpool,
            tc.tile_pool(name="v1", bufs=1) as vpool,
            tc.tile_pool(name="sc", bufs=NG) as spool,
            tc.tile_pool(name="osb", bufs=1) as opool,
            tc.tile_pool(name="pacc", bufs=1, space="PSUM") as paccp,
        ):
            m_sb = mpool.tile([P, NG * K], bf16, tag="m")
            bt_sb = btpool.tile([P, NG * B], bf16, tag="bt")
            ec = ecpool.tile([P, NG * 2 * K], bf16, tag="ec")
            v1t = vpool.tile([P, NG * B], bf16, tag="v1t")
            psum = paccp.tile([B, 2 * K], f32, tag="acc")

            scratch = vpool.tile([P, 2 * K], bf16, tag="warm")
            nc.gpsimd.memset(scratch[:], 0.0)
            wpsum = paccp.tile([B, 2 * K], f32, tag="warmps")
            for _ in range(N_WARM):
                nc.tensor.matmul(
                    wpsum[:], scratch[:, 0:B], scratch[:], start=True, stop=True
                )

            m3 = m_sb[:].rearrange("p (g k) -> p g k", g=NG)
            md3 = m_d.rearrange("p (g k) -> p g k", g=NG)
            bt3 = bt_sb[:].rearrange("p (g b) -> p g b", g=NG)
            ec3 = ec[:].rearrange("p (g k) -> p g k", g=NG)
            v3 = v1t[:].rearrange("p (g b) -> p g b", g=NG)

            # Input DMAs in pipeline order on the SP HWDGE ring (input
            # traffic shares 5 SDMA engines; strict FIFO per ring).  Small
            # first chunk starts the ACT chain early; bT rides second so
            # v1T never gates; per-group chunks keep completions flowing.
            nc.sync.dma_start(out=m3[:, 0:1, :], in_=md3[:, 0:1, :])
            nc.sync.dma_start(out=m3[:, 1:2, :], in_=md3[:, 1:2, :])
            nc.scalar.dma_start(out=m3[:, 2:3, :], in_=md3[:, 2:3, :])
            nc.sync.dma_start(out=bt_sb[:], in_=bt_d)
            nc.sync.dma_start(out=m3[:, 3:4, :], in_=md3[:, 3:4, :])
            nc.sync.dma_start(out=m3[:, 4:5, :], in_=md3[:, 4:5, :])

            for g in range(NG):
                # E_g (bf16) with fused row-sum accumulator s_g.
                s = spool.tile([P, 1], f32, tag="s")
                nc.scalar.activation(
                    ec3[:, g, 0:K], m3[:, g, :], Act.Exp, scale=-ALPHA,
                    accum_out=s[:],
                )
                # C_g = E_g * M_g.  GpSimd for early groups (parallel with
                # DVE); DVE for the last group, whose C is on the critical
                # path into the final matmul.
                ceng = nc.vector if g == NG - 2 else nc.gpsimd
                ceng.tensor_tensor(
                    ec3[:, g, K : 2 * K], ec3[:, g, 0:K], m3[:, g, :],
                    op=Alu.mult,
                )
                # r_g = 1/s_g; v1T_g = (bT_g * K) * r_g.  (The reference's
                # eps=1e-16 on K^T u0 >= 2e-9 is below f32 resolution of
                # the quotient -- dropped.  The 1/K on s folds into the
                # v1T scale.)
                r = spool.tile([P, 1], f32, tag="r")
                nc.vector.reciprocal(r[:], s[:])
                nc.vector.tensor_scalar(
                    v3[:, g, :], bt3[:, g, :], r[:], float(K),
                    op0=Alu.mult, op1=Alu.mult,
                )
                # [Kv1 | G] += v1T_g.T @ [E_g | C_g]
                nc.tensor.matmul(
                    psum[:], v3[:, g, :], ec3[:, g, :],
                    start=(g == 0), stop=(g == NG - 1),
                )
                if g in (1, 2):
                    # filler matmul: keeps the PE HAM clock gate lifted
                    # through the dependency gaps between real matmuls
                    nc.tensor.matmul(
                        wpsum[:], scratch[:, 0:B], scratch[:],
                        start=True, stop=True,
                    )

            out_sb = opool.tile([B, 2 * K], bf16, tag="osb")
            nc.vector.tensor_copy(out_sb[:], psum[:])
            nc.sync.dma_start(out=o_d, in_=out_sb[:])

    nc.compile()
    return nc


def _get_nc():
    if "nc" not in _CACHE:
        _CACHE["nc"] = _build_nc()
    return _CACHE["nc"]


def _shard_host(b, M):
    """Pre-arrange shards into the on-chip layout: group-fold v into
    [125, 5*...] with groups side by side in the free dimension, bf16."""
    import ml_dtypes

    M = np.asarray(M, dtype=np.float32)
    bt = np.asarray(b, dtype=np.float32).T.astype(ml_dtypes.bfloat16)  # [V, B]
    in_maps = []
    for c in range(NCORES):
        lo, hi = c * VC, (c + 1) * VC
        # [VC, K] -> [NG, P, K] -> [P, NG, K] -> [P, NG*K]
        msh = (
            M[lo:hi, :]
            .astype(ml_dtypes.bfloat16)
            .reshape(NG, P, K)
            .transpose(1, 0, 2)
            .reshape(P, NG * K)
        )
        bsh = (
            bt[lo:hi, :].reshape(NG, P, B).transpose(1, 0, 2).reshape(P, NG * B)
        )
        in_maps.append(
            {
                "m_sh": np.ascontiguousarray(msh),
                "bt_sh": np.ascontiguousarray(bsh),
            }
        )
    return in_maps


def run_on_hw(a, b, M, trace=False):
    """Returns (loss, BassKernelResults)."""
    from concourse import bass_utils

    nc = _get_nc()
    res = bass_utils.run_bass_kernel_spmd(
        nc,
        _shard_host(b, M),
        core_ids=list(range(NCORES)),
        trace=trace,
    )
    outs = [res.results[c]["out"] for c in range(NCORES)]
    acc = np.sum(np.stack(outs, axis=0), axis=0)  # [B, 2K]
    kv1 = acc[:, :K]
    g = acc[:, K:]
    u1 = np.asarray(a, dtype=np.float32) / (kv1 + np.float32(EPS))
    loss = np.float32(np.mean(np.sum(u1 * g, axis=1)))
    return np.asarray(loss), res


def kernel(a, b, M):
    loss, _ = run_on_hw(a, b, M, trace=False)
    return loss


# revision 25
# speedup vs baseline: 1.0071x; 1.0039x over previous
"""Trainium2 Bass kernel for nn_CTR_27754078666791 (batched Sinkhorn OT loss).

Reference semantics: 200-iteration Sinkhorn with a convergence check at
t = 0, 50, 100, 150 that freezes the iterates once
    max_b |sum_k u_new*Kv - sum_k a| <= 5e-3.
Because u_new = a/(Kv+eps), the checked quantity is a/(Kv+eps)*Kv ~ a up to
f32 rounding (~1e-4), so the check passes at t=0 for any inputs: the loop
always freezes after ONE Sinkhorn iteration from the uniform init
u0 = 1/K, v0 = 1/V.  The computation therefore reduces to:

    E[v,k]  = exp(-alpha*M[v,k])                  (K_mat transposed)
    s[v]    = sum_k E[v,k] / K                     (= K^T u0, batch-indep)
    v1[b,v] = b[b,v] / (s[v] + eps)
    Kv1     = v1 @ E          [B,K]
    G       = v1 @ (E*M)      [B,K]
    u1      = a / (Kv1 + eps)
    loss    = mean_b sum_k u1[b,k] * G[b,k]

Distribution: shard V=5000 across 8 cores (625 rows of M / cols of b each).
Each core reads only its M/b shard, produces partial Kv1_c / G_c [B,K]
sums; the tiny [64,256] partials are summed on host (the final mean
all-reduce), where u1 and the loss are formed.

Device kernel per core (Tile), final:
  - Host pre-arranges shards into the exact SBUF layout (625 v-rows folded
    into 5 groups of 125 partitions side by side in the free dim) and
    converts to bf16: M [125, 5*256], bT [125, 5*64] (b pre-transposed, so
    no on-device transposes).  bf16 M shifts the loss by ~2e-4 relative
    (verified) -- far inside the tolerance -- and halves the dominant DMA.
    DRAM->SBUF traffic is pinned to 5 SDMA engines (~120 GB/s), so input
    bytes are the scarce resource.
  - Input DMAs ride per-group: SP HWDGE ring carries g0, g1, bT, g3, g4
    in pipeline order while g2 goes alone on the ACT ring (independent
    completion queue, pulls the mid-chain gate ~1us earlier).  Per-chunk
    completion semaphores lag the data by 1-2 us under load, which is why
    chunk order and ring placement matter more than raw bandwidth.
  - Dummy matmuls on a zeroed scratch tile run during the DMA wait (plus
    two fillers mid-stream) to lift the PE HAM clock gate (cold PE runs
    at 1.2 GHz; warm at 2.4 GHz) before the real matmuls.
  - Per group g: ACT does E_g=exp(-20*M_g) with the row-sum fused into the
    activation accumulator (s_g); the E*M product C_g runs on GpSimd
    except group 3 on DVE (critical-path balance); DVE does
    r_g=1/s_g and v1T_g=(bT_g*K)*r_g; PE runs one bf16 matmul
    [Kv1_g|G_g] += v1T_g.T @ [E_g|C_g] into a single [64,512] PSUM bank.
  - One DVE cast PSUM->SBUF (bf16); one DMA back to DRAM; the host sums
    the 8 partials and finishes u1 and the mean loss.
"""

import numpy as np

# Problem constants (hardcoded per harness contract).
B = 64
K = 256
V = 5000
NCORES = 8
VC = V // NCORES  # 625 rows of M per core
P = 125           # partition rows per group
NG = VC // P      # 5 groups per core
GSPLIT = 2        # m DMA chunk A covers groups [0, GSPLIT)
N_WARM = 9         # dummy matmuls to lift the PE HAM clock gate during DMA
ALPHA = 20.0
EPS = 1e-16

_CACHE = {}


def _build_nc():
    from concourse import bacc, mybir, tile

    f32 = mybir.dt.float32
    bf16 = mybir.dt.bfloat16
    Act = mybir.ActivationFunctionType
    Alu = mybir.AluOpType

    nc = bacc.Bacc(
        "TRN2",
        debug=False,
        enable_asserts=False,
        num_devices=NCORES,
    )
    m_d = nc.dram_tensor("m_sh", [P, NG * K], bf16, kind="ExternalInput").ap()
    bt_d = nc.dram_tensor("bt_sh", [P, NG * B], bf16, kind="ExternalInput").ap()
    o_d = nc.dram_tensor("out", [B, 2 * K], bf16, kind="ExternalOutput").ap()

    with tile.TileContext(nc) as tc:
        with (
            tc.tile_pool(name="mt", bufs=1) as mpool,
            tc.tile_pool(name="bt", bufs=1) as btpool,
            tc.tile_pool(name="ec", bufs=1) as ecpool,
            tc.tile_pool(name="v1", bufs=1) as vpool,
            tc.tile_pool(name="sc", bufs=NG) as spool,
            tc.tile_pool(name="osb", bufs=1) as opool,
            tc.tile_pool(name="pacc", bufs=1, space="PSUM") as paccp,
        ):
            m_sb = mpool.tile([P, NG * K], bf16, tag="m")
            bt_sb = btpool.tile([P, NG * B], bf16, tag="bt")
            ec = ecpool.tile([P, NG * 2 * K], bf16, tag="ec")
            v1t = vpool.tile([P, NG * B], bf16, tag="v1t")
            psum = paccp.tile([B, 2 * K], f32, tag="acc")

            scratch = vpool.tile([P, 2 * K], bf16, tag="warm")
            nc.gpsimd.memset(scratch[:], 0.0)
            wpsum = paccp.tile([B, 2 * K], f32, tag="warmps")
            for _ in range(N_WARM):
                nc.tensor.matmul(
                    wpsum[:], scratch[:, 0:B], scratch[:], start=True, stop=True
                )

            m3 = m_sb[:].rearrange("p (g k) -> p g k", g=NG)
            md3 = m_d.rearrange("p (g k) -> p g k", g=NG)
            bt3 = bt_sb[:].rearrange("p (g b) -> p g b", g=NG)
            ec3 = ec[:].rearrange("p (g k) -> p g k", g=NG)
            v3 = v1t[:].rearrange("p (g b) -> p g b", g=NG)

            # Input DMAs in pipeline order on the SP HWDGE ring (input
            # traffic shares 5 SDMA engines; strict FIFO per ring).  Small
            # first chunk starts the ACT chain early; bT rides second so
            # v1T never gates; per-group chunks keep completions flowing.
            nc.sync.dma_start(out=m3[:, 0:1, :], in_=md3[:, 0:1, :])
            nc.sync.dma_start(out=m3[:, 1:2, :], in_=md3[:, 1:2, :])
            nc.scalar.dma_start(out=m3[:, 2:3, :], in_=md3[:, 2:3, :])
            nc.sync.dma_start(out=bt_sb[:], in_=bt_d)
            nc.sync.dma_start(out=m3[:, 3:4, :], in_=md3[:, 3:4, :])
            nc.sync.dma_start(out=m3[:, 4:5, :], in_=md3[:, 4:5, :])

            for g in range(NG):
                # E_g (bf16).  Groups 0-3 compute the row-sum on DVE in
                # parallel (keeps the ACT serial chain short, ~0.5us/group
                # instead of ~0.79); the last group keeps the fused ACT
                # accumulator (faster than a reduce and nothing queues
                # behind it).
                s = spool.tile([P, 1], f32, tag="s")
                if g == NG - 1:
                    nc.scalar.activation(
                        ec3[:, g, 0:K], m3[:, g, :], Act.Exp, scale=-ALPHA,
                        accum_out=s[:],
                    )
                else:
                    nc.scalar.activation(
                        ec3[:, g, 0:K], m3[:, g, :], Act.Exp, scale=-ALPHA,
                    )
                    nc.vector.tensor_reduce(
                        s[:], ec3[:, g, 0:K], axis=mybir.AxisListType.X,
                        op=Alu.add,
                    )
                # C_g = E_g * M_g.  GpSimd for early groups (parallel with
                # DVE); DVE for the last group, whose C is on the critical
                # path into the final matmul.
                ceng = nc.vector if g == NG - 2 else nc.gpsimd
                ceng.tensor_tensor(
                    ec3[:, g, K : 2 * K], ec3[:, g, 0:K], m3[:, g, :],
                    op=Alu.mult,
                )
                # r_g = 1/s_g; v1T_g = (bT_g * K) * r_g.  (The reference's
                # eps=1e-16 on K^T u0 >= 2e-9 is below f32 resolution of
                # the quotient -- dropped.  The 1/K on s folds into the
                # v1T scale.)
                r = spool.tile([P, 1], f32, tag="r")
                nc.vector.reciprocal(r[:], s[:])
                nc.vector.tensor_scalar(
                    v3[:, g, :], bt3[:, g, :], r[:], float(K),
                    op0=Alu.mult, op1=Alu.mult,
                )
                # [Kv1 | G] += v1T_g.T @ [E_g | C_g]
                nc.tensor.matmul(
                    psum[:], v3[:, g, :], ec3[:, g, :],
                    start=(g == 0), stop=(g == NG - 1),
                )
                if g in (1, 2):
                    # filler matmul: keeps the PE HAM clock gate lifted
                    # through the dependency gaps between real matmuls
                    nc.tensor.matmul(
                        wpsum[:], scratch[:, 0:B], scratch[:],
                        start=True, stop=True,
                    )

            out_sb = opool.tile([B, 2 * K], bf16, tag="osb")
            nc.vector.tensor_copy(out_sb[:], psum[:])
            nc.sync.dma_start(out=o_d, in_=out_sb[:])

    nc.compile()
    return nc


def _get_nc():
    if "nc" not in _CACHE:
        _CACHE["nc"] = _build_nc()
    return _CACHE["nc"]


def _shard_host(b, M):
    """Pre-arrange shards into the on-chip layout: group-fold v into
    [125, 5*...] with groups side by side in the free dimension, bf16."""
    import ml_dtypes

    M = np.asarray(M, dtype=np.float32)
    bt = np.asarray(b, dtype=np.float32).T.astype(ml_dtypes.bfloat16)  # [V, B]
    in_maps = []
    for c in range(NCORES):
        lo, hi = c * VC, (c + 1) * VC
        # [VC, K] -> [NG, P, K] -> [P, NG, K] -> [P, NG*K]
        msh = (
            M[lo:hi, :]
            .astype(ml_dtypes.bfloat16)
            .reshape(NG, P, K)
            .transpose(1, 0, 2)
            .reshape(P, NG * K)
        )
        bsh = (
            bt[lo:hi, :].reshape(NG, P, B).transpose(1, 0, 2).reshape(P, NG * B)
        )
        in_maps.append(
            {
                "m_sh": np.ascontiguousarray(msh),
                "bt_sh": np.ascontiguousarray(bsh),
            }
        )
    return in_maps


def run_on_hw(a, b, M, trace=False):
    """Returns (loss, BassKernelResults)."""
    from concourse import bass_utils

    nc = _get_nc()
    res = bass_utils.run_bass_kernel_spmd(
        nc,
        _shard_host(b, M),
        core_ids=list(range(NCORES)),
        trace=trace,
    )
    outs = [res.results[c]["out"] for c in range(NCORES)]
    acc = np.sum(np.stack(outs, axis=0), axis=0)  # [B, 2K]
    kv1 = acc[:, :K]
    g = acc[:, K:]
    u1 = np.asarray(a, dtype=np.float32) / (kv1 + np.float32(EPS))
    loss = np.float32(np.mean(np.sum(u1 * g, axis=1)))
    return np.asarray(loss), res


def kernel(a, b, M):
    loss, _ = run_on_hw(a, b, M, trace=False)
    return loss
